# revision 1
# baseline (speedup 1.0000x reference)
"""Trainium2 Bass kernel for nn_CausalSelfAttention (erf-kernel attention).

Sharding: 8 cores = 2 batches x 4 core-groups; each core handles one batch
and 3 of the 12 heads (data-parallel over batch, head-parallel within batch).
Each core computes its 3 heads' full attention plus its partial output
projection; the host sums the 4 partials per batch.

Device-side layout strategy (per core):
  - x arrives pre-transposed from host: xT [768, 2048] (contract dim on
    partitions for the QKV matmuls), in the matmul storage dtype.
  - Host packs/permutes QKV weight rows into one [768, 576] matrix "wall"
    whose 5 output chunks of <=128 land directly in the SBUF row layout the
    rest of the kernel wants:
        C1 = [q_h0 | q_h1]   (rope-permuted rows: even dims then odd dims)
        C2 = [k_h0 | k_h1]
        C3 = [v_h0 | q_h2]
        C4 = [v_h1 | k_h2]
        C5 = [v_h2]
    The even/odd permutation makes RoPE operate on contiguous 32-partition
    blocks; scores are invariant to a shared q/k head-dim permutation.
  - RoPE: partner swap via a PE permutation matmul, then 3 DVE ops in fp32,
    writing rotated q/k into fresh tiles of the matmul dtype.
  - Scores computed transposed: sT[t, s] = kT.T @ qT per (128 t x 512 s)
    tile, causal tiles only.  erf(0.125*x) on ACT, +1 on DVE, diagonal
    band masked with affine_select on Pool.
  - AV: yT[d, s] accumulated in PSUM over t-chunks with v_ext [t, 65]
    stationary (65th column of ones produces the softmax-denominator row).
  - Normalization: reciprocal of denom row, replicated across partitions by
    a K=1 matmul, multiplied into yT.
  - Projection: out[s, e] = yT.T @ WprojT per head, PSUM-accumulated across
    heads, copied to SBUF and DMA'd to DRAM.

Matmul storage dtype (KERNEL_DTYPE): fp32 (4 cyc/row, exact), f32r
(1 cyc/row at N>=256, reduced mantissa), bf16 (1 cyc/row, 2-byte storage).
PSUM accumulation is always fp32.
"""

import os
import sys
from contextlib import ExitStack

import numpy as np

for _p in ("/opt/trn_rl_repo",):
    if _p not in sys.path:
        sys.path.insert(0, _p)

import concourse.bass as bass
import concourse.mybir as mybir
from concourse.bass_utils import run_bass_kernel_spmd
from concourse.tile import TileContext

S = 2048          # sequence length per batch
D = 768           # model dim
HD = 64           # head dim
HPC = 3           # heads per core
NCORES = 8
F32 = mybir.dt.float32
NT = S // 512     # 4 free-dim tiles of 512
TC = S // 128     # 16 t-chunks of 128
EPS = 1e-6

DTYPE_NAME = os.environ.get("KERNEL_DTYPE", "f32r")
IN_DT = {
    "fp32": mybir.dt.float32,
    "f32r": mybir.dt.float32r,
    "bf16": mybir.dt.bfloat16,
}[DTYPE_NAME]
# CoreSim doesn't implement Erf; dev-only switch to validate logic in sim.
ERF_FUNC_NAME = "Tanh" if os.environ.get("KERNEL_SIM_TANH", "0") == "1" else "Erf"

LAST_EXEC_NS = None
LAST_RESULTS = None


def _split_multi_waits(nc: bass.Bass) -> None:
    """This walrus build rejects instructions carrying more than one sync
    wait (codegen 'Too many sync wait commands', hit by the Tile kernel-tail
    drain).  Hoist all but the last wait of any multi-wait instruction onto
    single-wait Drain instructions inserted just before it on the same
    engine — semantically identical, one wait per instruction."""
    for f in nc.m.functions:
        for b in f.blocks:
            new_insts = []
            changed = False
            for inst in b.instructions:
                si = inst.sync_info
                waits = list(si.on_wait) if si is not None and si.on_wait else []
                if len(waits) > 1:
                    changed = True
                    for n, w in enumerate(waits[:-1]):
                        d = mybir.InstDrain(
                            name=f"{inst.name}-wsplit{n}",
                            engine=inst.engine,
                            ins=[],
                            outs=[],
                            sync_info=mybir.SyncInfo(on_wait=[w], on_update=[]),
                        )
                        new_insts.append(d)
                    si.on_wait = [waits[-1]]
                new_insts.append(inst)
            if changed:
                b.instructions[:] = new_insts


def build_program() -> bass.Bass:
    nc = bass.Bass(target_bir_lowering=False, debug=False)

    x_t = nc.declare_dram_parameter("xt", [D, S], IN_DT, isOutput=False)
    wall = nc.declare_dram_parameter("wall", [D, 576], IN_DT, isOutput=False)
    wproj = nc.declare_dram_parameter("wproj", [HPC * HD, D], IN_DT, isOutput=False)
    csc = nc.declare_dram_parameter("csc", [128, S], F32, isOutput=False)
    css = nc.declare_dram_parameter("css", [128, S], F32, isOutput=False)
    swp = nc.declare_dram_parameter("swp", [128, 128], IN_DT, isOutput=False)
    iden = nc.declare_dram_parameter("iden", [128, 128], F32, isOutput=False)
    out_d = nc.declare_dram_parameter("out", [S, D], F32, isOutput=True)

    with TileContext(nc) as tc:
        with ExitStack() as ctx:
            const = ctx.enter_context(tc.tile_pool(name="const", bufs=1))
            big = ctx.enter_context(tc.tile_pool(name="big", bufs=10))
            wpool = ctx.enter_context(tc.tile_pool(name="wpool", bufs=3))
            tpool = ctx.enter_context(tc.tile_pool(name="tpool", bufs=2))
            npool = ctx.enter_context(tc.tile_pool(name="npool", bufs=2))
            ps_a = ctx.enter_context(tc.tile_pool(name="ps_a", bufs=3, space="PSUM"))
            ps_s = ctx.enter_context(tc.tile_pool(name="ps_s", bufs=2, space="PSUM"))
            ps_y = ctx.enter_context(tc.tile_pool(name="ps_y", bufs=2, space="PSUM"))
            ps_r = ctx.enter_context(tc.tile_pool(name="ps_r", bufs=1, space="PSUM"))

            # ---- constants / inputs ----
            XT = []
            for kc in range(6):
                t = big.tile([128, S], IN_DT, tag="big", name=f"xt{kc}")
                nc.sync.dma_start(out=t, in_=x_t[kc * 128:(kc + 1) * 128, :])
                XT.append(t)
            WA = []
            for kc in range(6):
                t = const.tile([128, 576], IN_DT, tag=f"wa{kc}", name=f"wa{kc}")
                nc.sync.dma_start(out=t, in_=wall[kc * 128:(kc + 1) * 128, :])
                WA.append(t)
            WP = []
            for h in range(HPC):
                t = const.tile([HD, D], IN_DT, tag=f"wp{h}", name=f"wp{h}")
                nc.sync.dma_start(out=t, in_=wproj[h * HD:(h + 1) * HD, :])
                WP.append(t)
            CSC = const.tile([128, S], F32, tag="csc")
            nc.sync.dma_start(out=CSC, in_=csc[:, :])
            CSS = const.tile([128, S], F32, tag="css")
            nc.sync.dma_start(out=CSS, in_=css[:, :])
            SWP = const.tile([128, 128], IN_DT, tag="swp")
            nc.sync.dma_start(out=SWP, in_=swp[:, :])
            ID = const.tile([128, 128], F32, tag="iden")
            nc.sync.dma_start(out=ID, in_=iden[:, :])
            ONESF = const.tile([128, HD], F32, tag="onesf")
            nc.vector.memset(ONESF, 1.0)
            ONES = const.tile([128, HD], IN_DT, tag="ones")
            nc.vector.tensor_copy(out=ONES, in_=ONESF)

            # ---- QKV projection: packed q/k/v row chunks ----
            # C1, C2 (pure q/k) rotate through the big pool — freed after
            # RoPE.  C3, C4, C5 carry v rows for the whole kernel -> const.
            C1 = big.tile([128, S], F32, tag="big", name="c1")
            C2 = big.tile([128, S], F32, tag="big", name="c2")
            C3 = const.tile([128, S], F32, tag="c3")
            C4 = const.tile([128, S], F32, tag="c4")
            C5 = const.tile([64, S], F32, tag="c5")
            RAW = [C1, C2, C3, C4, C5]
            for m in range(5):
                msz = 128 if m < 4 else 64
                for nt in range(NT):
                    ns = slice(nt * 512, (nt + 1) * 512)
                    ps = ps_a.tile([128, 512], F32, tag="ps_a")
                    for kc in range(6):
                        nc.tensor.matmul(
                            ps[:msz, :],
                            lhsT=WA[kc][:, m * 128:m * 128 + msz],
                            rhs=XT[kc][:, ns],
                            start=(kc == 0),
                            stop=(kc == 5),
                        )
                    nc.vector.tensor_copy(out=RAW[m][:, ns], in_=ps[:msz, :])

            # ---- RoPE: rotate q/k rows into fresh IN_DT tiles ----
            # rows [r0, r0+64) hold one head's [even dims | odd dims]; the
            # partner value (odd for even rows, even for odd rows) comes from
            # a PE swap matmul; csc/css carry cos and sign-baked sin.
            QF = big.tile([128, S], IN_DT, tag="big", name="qf")
            KF = big.tile([128, S], IN_DT, tag="big", name="kf")
            Q2F = big.tile([128, S], IN_DT, tag="big", name="q2f")
            K2F = big.tile([128, S], IN_DT, tag="big", name="k2f")

            def rope(raw, out, r0, rsz):
                # The swap matmul always runs on all 128 rows with output at
                # partition 0 (f32r matmuls require dst partition 0; extra
                # rows cost nothing — matmul time is column count).  The
                # rotate ops then slice the rows they need, base-aligned.
                nrows = raw.shape[0]
                for nt in range(NT):
                    ns = slice(nt * 512, (nt + 1) * 512)
                    rs = slice(r0, r0 + rsz)
                    if IN_DT != F32:
                        # swap-matmul needs an IN_DT rhs produced by a
                        # rounding op (the BIR verifier rejects bitcasts
                        # into f32r): convert via a copy first
                        swin = tpool.tile([128, 512], IN_DT, tag="cv")
                        nc.vector.tensor_copy(out=swin[:nrows, :], in_=raw[:, ns])
                        swin_ap = swin[:nrows, :]
                    else:
                        swin_ap = raw[:, ns]
                    sw = ps_a.tile([128, 512], F32, tag="ps_a")
                    nc.tensor.matmul(
                        sw[:nrows, :],
                        lhsT=SWP[:nrows, :nrows],
                        rhs=swin_ap,
                        start=True,
                        stop=True,
                    )
                    t1 = tpool.tile([128, 512], F32, tag="t1")
                    t2 = tpool.tile([128, 512], F32, tag="t2")
                    nc.vector.tensor_mul(t1[rs, :], raw[rs, ns], CSC[rs, ns])
                    nc.vector.tensor_mul(t2[rs, :], sw[rs, :], CSS[rs, ns])
                    nc.vector.tensor_add(out[rs, ns], t1[rs, :], t2[rs, :])

            rope(C1, QF, 0, 128)     # q_h0, q_h1
            rope(C2, KF, 0, 128)     # k_h0, k_h1
            rope(C3, Q2F, 64, 64)    # q_h2 (rows 64:128; rows 0:64 are v_h0)
            rope(C4, K2F, 64, 64)    # k_h2

            # ---- v_ext[h]: 16 chunks of [128 t, 65] = [v^T chunk | ones] ----
            VSRC = [C3[0:64, :], C4[0:64, :], C5[0:64, :]]
            VEXT = []
            for h in range(HPC):
                ve = big.tile([128, TC * 65], IN_DT, tag="big", name=f"vext{h}")
                VEXT.append(ve)
            for h in range(HPC):
                # ones into every 65th column (the denominator generator)
                ve3 = VEXT[h].rearrange("p (t c) -> p t c", c=65)
                nc.vector.tensor_copy(out=ve3[:, :, 64], in_=ONESF[:, 0:TC])
                for tcb in range(TC):
                    pt = ps_a.tile([128, 512], F32, tag="ps_a")
                    nc.tensor.transpose(
                        pt[:, 0:HD],
                        in_=VSRC[h][:, tcb * 128:(tcb + 1) * 128],
                        identity=ID[0:HD, 0:HD],
                    )
                    nc.vector.tensor_copy(
                        out=VEXT[h][:, tcb * 65:tcb * 65 + HD], in_=pt[:, 0:HD]
                    )

            # ---- attention per head ----
            QSRC = [QF[0:64, :], QF[64:128, :], Q2F[64:128, :]]
            KSRC = [KF[0:64, :], KF[64:128, :], K2F[64:128, :]]
            YT = []
            for h in range(HPC):
                YT.append(big.tile([HD, S], IN_DT, tag="big", name=f"yt{h}"))

            for h in range(HPC):
                q, k = QSRC[h], KSRC[h]
                for si in range(NT):
                    ss = slice(si * 512, (si + 1) * 512)
                    ntc = 4 * (si + 1)
                    yps = ps_y.tile([65, 512], F32, tag="ps_y")
                    for tcb in range(ntc):
                        sc = ps_s.tile([128, 512], F32, tag="ps_s")
                        nc.tensor.matmul(
                            sc,
                            lhsT=k[:, tcb * 128:(tcb + 1) * 128],
                            rhs=q[:, ss],
                            start=True,
                            stop=True,
                        )
                        wt = wpool.tile([128, 512], IN_DT, tag="wt")
                        nc.scalar.activation(
                            out=wt, in_=sc,
                            func=getattr(mybir.ActivationFunctionType, ERF_FUNC_NAME),
                            scale=0.125,
                        )
                        nc.vector.tensor_scalar_add(wt, wt, 1.0)
                        if tcb >= 4 * si:
                            # diagonal band: zero the t > s corner
                            nc.gpsimd.affine_select(
                                out=wt, in_=wt,
                                compare_op=mybir.AluOpType.is_ge,
                                fill=0.0,
                                base=si * 512 - tcb * 128,
                                channel_multiplier=-1,
                                pattern=[[1, 512]],
                            )
                        nc.tensor.matmul(
                            yps,
                            lhsT=VEXT[h][:, tcb * 65:(tcb + 1) * 65],
                            rhs=wt,
                            start=(tcb == 0),
                            stop=(tcb == ntc - 1),
                        )
                    # normalize: yT[0:64] / max(denom row, eps)
                    dmx = npool.tile([65, 512], F32, tag="dmx")
                    nc.vector.tensor_scalar_max(dmx[64:65, :], yps[64:65, :], EPS)
                    rcpf = npool.tile([65, 512], F32, tag="rcpf")
                    nc.vector.reciprocal(rcpf[64:65, :], dmx[64:65, :])
                    rcp = npool.tile([65, 512], IN_DT, tag="rcp")
                    nc.vector.tensor_copy(out=rcp[64:65, :], in_=rcpf[64:65, :])
                    rep = ps_r.tile([HD, 512], F32, tag="ps_r")
                    nc.tensor.matmul(
                        rep,
                        lhsT=ONES[64:65, 0:HD],
                        rhs=rcp[64:65, :],
                        start=True,
                        stop=True,
                    )
                    rsb = npool.tile([HD, 512], F32, tag="rsb")
                    nc.vector.tensor_copy(out=rsb, in_=rep)
                    nc.vector.tensor_mul(YT[h][:, ss], yps[0:64, :], rsb)

            # ---- output projection (partial over this core's heads) ----
            for sci in range(TC):
                scs = slice(sci * 128, (sci + 1) * 128)
                po1 = ps_a.tile([128, 512], F32, tag="ps_a")
                po2 = ps_a.tile([128, 512], F32, tag="ps_a")
                for h in range(HPC):
                    nc.tensor.matmul(
                        po1,
                        lhsT=YT[h][:, scs],
                        rhs=WP[h][:, 0:512],
                        start=(h == 0),
                        stop=(h == HPC - 1),
                    )
                    nc.tensor.matmul(
                        po2[:, 0:256],
                        lhsT=YT[h][:, scs],
                        rhs=WP[h][:, 512:768],
                        start=(h == 0),
                        stop=(h == HPC - 1),
                    )
                ost = tpool.tile([128, D], F32, tag="ost", bufs=3)
                if sci % 2 == 0:
                    nc.scalar.copy(out=ost[:, 0:512], in_=po1)
                    nc.vector.tensor_copy(out=ost[:, 512:768], in_=po2[:, 0:256])
                else:
                    nc.vector.tensor_copy(out=ost[:, 0:512], in_=po1)
                    nc.scalar.copy(out=ost[:, 512:768], in_=po2[:, 0:256])
                nc.sync.dma_start(out=out_d[scs, :], in_=ost)

    return nc


_PROGRAM = None


def _get_program() -> bass.Bass:
    global _PROGRAM
    if _PROGRAM is None:
        _PROGRAM = build_program()
        _split_multi_waits(_PROGRAM)
    return _PROGRAM


def _np_indt(arr):
    return np.ascontiguousarray(arr).astype(mybir.dt.np(IN_DT))


def make_in_maps(x, Wq, Wk, Wv, Wproj):
    x = np.asarray(x, dtype=np.float32)
    Wq = np.asarray(Wq, dtype=np.float32)
    Wk = np.asarray(Wk, dtype=np.float32)
    Wv = np.asarray(Wv, dtype=np.float32)
    Wproj = np.asarray(Wproj, dtype=np.float32)

    half = HD // 2
    j = np.arange(half, dtype=np.float64)
    freq = 1.0 / (10000.0 ** (j / half))
    ang = np.arange(S, dtype=np.float64)[None, :] * freq[:, None]   # [32, S]
    cosT = np.cos(ang).astype(np.float32)
    sinT = np.sin(ang).astype(np.float32)
    csc = np.tile(np.vstack([cosT, cosT]), (2, 1))                  # [128, S]
    css = np.tile(np.vstack([-sinT, sinT]), (2, 1))

    swp = np.zeros((128, 128), dtype=np.float32)
    for blk in range(2):
        for jj in range(half):
            swp[blk * 64 + jj, blk * 64 + half + jj] = 1.0
            swp[blk * 64 + half + jj, blk * 64 + jj] = 1.0
    iden = np.eye(128, dtype=np.float32)

    perm = np.concatenate([np.arange(0, HD, 2), np.arange(1, HD, 2)])

    in_maps = []
    for c in range(NCORES):
        b = c // 4
        hs = [(c % 4) * HPC + i for i in range(HPC)]
        rq = [Wq[h * HD:(h + 1) * HD][perm, :] for h in hs]
        rk = [Wk[h * HD:(h + 1) * HD][perm, :] for h in hs]
        rv = [Wv[h * HD:(h + 1) * HD, :] for h in hs]
        cols = np.concatenate(
            [rq[0], rq[1], rk[0], rk[1], rv[0], rq[2], rv[1], rk[2], rv[2]],
            axis=0,
        )                                                           # [576, D]
        wall = np.ascontiguousarray(cols.T)                         # [D, 576]
        dims = np.concatenate([np.arange(h * HD, (h + 1) * HD) for h in hs])
        wproj_t = np.ascontiguousarray(Wproj[:, dims].T)            # [192, D]
        in_maps.append({
            "xt": _np_indt(x[b].T),
            "wall": _np_indt(wall),
            "wproj": _np_indt(wproj_t),
            "csc": csc,
            "css": css,
            "swp": _np_indt(swp),
            "iden": iden,
        })
    return in_maps


def kernel(x, Wq, Wk, Wv, Wproj):
    global LAST_EXEC_NS, LAST_RESULTS
    nc = _get_program()
    in_maps = make_in_maps(x, Wq, Wk, Wv, Wproj)
    trace = os.environ.get("KERNEL_TRACE", "0") == "1"
    res = run_bass_kernel_spmd(nc, in_maps, list(range(NCORES)), trace=trace)
    LAST_EXEC_NS = res.exec_time_ns
    LAST_RESULTS = res
    outs = [np.asarray(r["out"], dtype=np.float32) for r in res.results]
    out = np.empty((2, S, D), dtype=np.float32)
    out[0] = outs[0] + outs[1] + outs[2] + outs[3]
    out[1] = outs[4] + outs[5] + outs[6] + outs[7]
    return out



# revision 7
# speedup vs baseline: 1.2427x; 1.2427x over previous
"""Trainium2 Bass kernel for nn_CausalSelfAttention (erf-kernel attention).

Sharding: 8 cores = 2 batches x 4 core-groups; each core handles one batch
and 3 of the 12 heads.  Each core computes its 3 heads' attention plus its
partial output projection; the host sums the 4 partials per batch.

v2 design (bf16 storage, fp32 PSUM accumulation):
  - Host packs q/k weight rows (rope-permuted: even dims then odd dims) into
    a [768, 384] "wall" (chunks: [q0|q1], [k0|k1], [q2|k2]) plus v rows
    [768, 192].  q/k computed via 3x4x6 matmuls into C tiles; v computed
    DIRECTLY TRANSPOSED per 128-token chunk (lhsT = xT chunk, rhs = WvT)
    so no PE transposes are needed.
  - v lands in VEXT [128, 16*384]: per chunk c, head h the 128 columns
    [v_h (64) | ones (64)].  The ones columns make the AV matmul emit the
    softmax denominator replicated across 64 partitions for free (M=128).
  - RoPE: partner swap via PE permutation matmul, then 3 DVE ops in bf16.
  - Scores per (si, t-chunk): [128 t, 512 s] tiles, causal chunks only,
    PAIRED into [128, 1024] PSUM regions so erf runs once per pair.
    weights = erf(0.125*scores) + 1 (ACT erf, DVE +1), diagonal band
    masked with affine_select on Pool.
  - AV accumulates yps [128, 512]: rows 0:64 = y, rows 64:128 = denom
    (identical).  reciprocal_approx_fast on rows 64:128, small SBUF->SBUF
    DMA realigns the reciprocal rows to 0:64 (compute engines are
    lane-locked; DMA moves partitions freely), one DVE mul normalizes.
  - Projection per 128-token chunk, PSUM-accumulated over 3 heads,
    copied to SBUF (ACT/Pool) and DMA'd out in fp32.
"""

import os
import sys
from contextlib import ExitStack

import numpy as np

for _p in ("/opt/trn_rl_repo",):
    if _p not in sys.path:
        sys.path.insert(0, _p)

import concourse.bass as bass
import concourse.mybir as mybir
from concourse.bass_utils import run_bass_kernel_spmd
from concourse.tile import TileContext

S = 2048          # sequence length per batch
D = 768           # model dim
HD = 64           # head dim
HPC = 3           # heads per core
NCORES = 8
F32 = mybir.dt.float32
BF16 = mybir.dt.bfloat16
NT = S // 512     # 4 free-dim tiles of 512
TC = S // 128     # 16 t-chunks of 128

# CoreSim doesn't implement Erf; dev-only switch to validate logic in sim.
ERF_FUNC_NAME = "Tanh" if os.environ.get("KERNEL_SIM_TANH", "0") == "1" else "Erf"

LAST_EXEC_NS = None
LAST_RESULTS = None


def _split_multi_waits(nc: bass.Bass) -> None:
    """This walrus build rejects instructions carrying more than one sync
    wait (codegen 'Too many sync wait commands', hit by the Tile kernel-tail
    drain).  Hoist all but the last wait of any multi-wait instruction onto
    single-wait Drain instructions inserted just before it on the same
    engine — semantically identical, one wait per instruction."""
    for f in nc.m.functions:
        for b in f.blocks:
            new_insts = []
            changed = False
            for inst in b.instructions:
                si = inst.sync_info
                waits = list(si.on_wait) if si is not None and si.on_wait else []
                if len(waits) > 1:
                    changed = True
                    for n, w in enumerate(waits[:-1]):
                        d = mybir.InstDrain(
                            name=f"{inst.name}-wsplit{n}",
                            engine=inst.engine,
                            ins=[],
                            outs=[],
                            sync_info=mybir.SyncInfo(on_wait=[w], on_update=[]),
                        )
                        new_insts.append(d)
                    si.on_wait = [waits[-1]]
                new_insts.append(inst)
            if changed:
                b.instructions[:] = new_insts


def build_program() -> bass.Bass:
    nc = bass.Bass(target_bir_lowering=False, debug=False)

    x_t = nc.declare_dram_parameter("xt", [D, S], BF16, isOutput=False)
    wall = nc.declare_dram_parameter("wall", [D, 576], BF16, isOutput=False)
    wproj = nc.declare_dram_parameter("wproj", [HPC * HD, D], BF16, isOutput=False)
    csc = nc.declare_dram_parameter("csc", [128, S], BF16, isOutput=False)
    css = nc.declare_dram_parameter("css", [128, S], BF16, isOutput=False)
    swp = nc.declare_dram_parameter("swp", [128, 128], BF16, isOutput=False)
    out_d = nc.declare_dram_parameter("out", [S, D], F32, isOutput=True)

    erf_func = getattr(mybir.ActivationFunctionType, ERF_FUNC_NAME)

    with TileContext(nc) as tc:
        with ExitStack() as ctx:
            const = ctx.enter_context(tc.tile_pool(name="const", bufs=1))
            tpool = ctx.enter_context(tc.tile_pool(name="tpool", bufs=3))
            wtp = ctx.enter_context(tc.tile_pool(name="wtp", bufs=3))
            npool = ctx.enter_context(tc.tile_pool(name="npool", bufs=2))
            ostp = ctx.enter_context(tc.tile_pool(name="ostp", bufs=3))

            # ---- inputs ----
            WA = []
            for kc in range(6):
                t = const.tile([128, 576], BF16, tag=f"wa{kc}", name=f"wa{kc}")
                nc.sync.dma_start(out=t, in_=wall[kc * 128:(kc + 1) * 128, :])
                WA.append(t)
            XT = []
            for kc in range(6):
                t = const.tile([128, S], BF16, tag=f"xt{kc}", name=f"xt{kc}")
                XT.append(t)
            # nt-major DMA order so the first QKV matmuls start early
            for ntb in range(NT):
                ns = slice(ntb * 512, (ntb + 1) * 512)
                for kc in range(6):
                    nc.sync.dma_start(out=XT[kc][:, ns], in_=x_t[kc * 128:(kc + 1) * 128, ns])
            CSC = const.tile([128, S], BF16, tag="csc")
            nc.sync.dma_start(out=CSC, in_=csc[:, :])
            CSS = const.tile([128, S], BF16, tag="css")
            nc.sync.dma_start(out=CSS, in_=css[:, :])
            SWP = const.tile([128, 128], BF16, tag="swp")
            nc.sync.dma_start(out=SWP, in_=swp[:, :])
            WP = []
            for h in range(HPC):
                t = const.tile([HD, D], BF16, tag=f"wp{h}", name=f"wp{h}")
                nc.sync.dma_start(out=t, in_=wproj[h * HD:(h + 1) * HD, :])
                WP.append(t)

            # VEXT: per chunk c, head h: cols c*384 + h*128 + [v(64) | ones(64)]
            VEXT = const.tile([128, TC * 384], BF16, tag="vext")
            v4 = VEXT.rearrange("p (c h x) -> p c h x", c=TC, h=HPC, x=128)
            for h in range(HPC):
                nc.gpsimd.memset(v4[:, :, h, HD:128], 1.0)

            # persistent q/k tiles (raw then roped)
            C = [const.tile([128, S], BF16, tag=f"c{m}", name=f"c{m}") for m in range(3)]
            R = [const.tile([128, S], BF16, tag=f"r{m}", name=f"r{m}") for m in range(3)]
            # k2 relocated to base partition 0 (matmul requires lhsT/rhs at
            # the same base partition; q2 is at rows 0:64 of R[2])
            K2 = const.tile([HD, S], BF16, tag="k2")
            YT = [const.tile([HD, S], BF16, tag=f"yt{h}", name=f"yt{h}") for h in range(HPC)]

            # ---- phase 1: QKV + RoPE (own PSUM scope) ----
            with tc.tile_pool(name="qkps", bufs=2, space="PSUM") as qkps, \
                 tc.tile_pool(name="vtps", bufs=2, space="PSUM") as vtps, \
                 tc.tile_pool(name="swps", bufs=2, space="PSUM") as swps:
                # q/k wall matmuls: C[m] rows = [q0|q1], [k0|k1], [q2|k2]
                for m in range(3):
                    for ntb in range(NT):
                        ns = slice(ntb * 512, (ntb + 1) * 512)
                        ps = qkps.tile([128, 512], F32, tag="qk")
                        for kc in range(6):
                            nc.tensor.matmul(
                                ps,
                                lhsT=WA[kc][:, m * 128:(m + 1) * 128],
                                rhs=XT[kc][:, ns],
                                start=(kc == 0),
                                stop=(kc == 5),
                            )
                        nc.vector.tensor_copy(out=C[m][:, ns], in_=ps)

                # v directly transposed: per t-chunk, [128 t, 192 d]
                for tcb in range(TC):
                    ts = slice(tcb * 128, (tcb + 1) * 128)
                    ps = vtps.tile([128, 512], F32, tag="vt")
                    for kc in range(6):
                        nc.tensor.matmul(
                            ps[:, 0:HPC * HD],
                            lhsT=XT[kc][:, ts],
                            rhs=WA[kc][:, 384:576],
                            start=(kc == 0),
                            stop=(kc == 5),
                        )
                    src = ps[:, 0:HPC * HD].rearrange("p (h x) -> p h x", h=HPC, x=HD)
                    nc.vector.tensor_copy(out=v4[:, tcb, :, 0:HD], in_=src)

                # RoPE: R[m] = C[m]*csc + swap(C[m])*css
                for ntb in range(NT):
                    ns = slice(ntb * 512, (ntb + 1) * 512)
                    for m in range(3):
                        sw = swps.tile([128, 512], F32, tag="sw")
                        nc.tensor.matmul(
                            sw, lhsT=SWP, rhs=C[m][:, ns], start=True, stop=True
                        )
                        t1 = tpool.tile([128, 512], BF16, tag="t1")
                        t2 = tpool.tile([128, 512], BF16, tag="t2")
                        nc.vector.tensor_mul(t1, C[m][:, ns], CSC[:, ns])
                        nc.vector.tensor_mul(t2, sw, CSS[:, ns])
                        nc.vector.tensor_add(R[m][:, ns], t1, t2)
                        if m == 2:
                            nc.sync.dma_start(out=K2[:, ns], in_=R[2][HD:128, ns])

            QS = [R[0][0:HD, :], R[0][HD:128, :], R[2][0:HD, :]]
            KS = [R[1][0:HD, :], R[1][HD:128, :], K2[:, :]]

            # ---- phase 2: attention + projection ----
            with tc.tile_pool(name="scps", bufs=2, space="PSUM") as scps, \
                 tc.tile_pool(name="ypsp", bufs=2, space="PSUM") as ypsp, \
                 tc.tile_pool(name="pop", bufs=2, space="PSUM") as pop:
                for si in range(NT):
                    ss = slice(si * 512, (si + 1) * 512)
                    for h in range(HPC):
                        npair = 2 * (si + 1)
                        yps = ypsp.tile([128, 512], F32, tag="yps")
                        for p in range(npair):
                            sc = scps.tile([128, 1024], F32, tag="sc")
                            for half in range(2):
                                tcb = 2 * p + half
                                nc.tensor.matmul(
                                    sc[:, half * 512:(half + 1) * 512],
                                    lhsT=KS[h][:, tcb * 128:(tcb + 1) * 128],
                                    rhs=QS[h][:, ss],
                                    start=True,
                                    stop=True,
                                )
                            wt = wtp.tile([128, 1024], BF16, tag="wt")
                            nc.scalar.activation(out=wt, in_=sc, func=erf_func, scale=0.125)
                            nc.vector.tensor_scalar_add(wt, wt, 1.0)
                            if p >= 2 * si:  # diagonal band pair
                                for half in range(2):
                                    tcb = 2 * p + half
                                    nc.gpsimd.affine_select(
                                        out=wt[:, half * 512:(half + 1) * 512],
                                        in_=wt[:, half * 512:(half + 1) * 512],
                                        compare_op=mybir.AluOpType.is_ge,
                                        fill=0.0,
                                        base=si * 512 - tcb * 128,
                                        channel_multiplier=-1,
                                        pattern=[[1, 512]],
                                    )
                            for half in range(2):
                                tcb = 2 * p + half
                                nc.tensor.matmul(
                                    yps,
                                    lhsT=VEXT[:, tcb * 384 + h * 128:tcb * 384 + (h + 1) * 128],
                                    rhs=wt[:, half * 512:(half + 1) * 512],
                                    start=(tcb == 0),
                                    stop=(tcb == 2 * npair - 1),
                                )
                        # normalize: rows 64:128 of yps hold denom (replicated).
                        # 1/d = exp(-ln(d)) on ACT — the custom-DVE reciprocal
                        # ops fail codegen on this walrus build, and plain DVE
                        # reciprocal is ~3.3us per call.
                        lg = npool.tile([128, 512], F32, tag="lg")
                        nc.scalar.activation(
                            out=lg[HD:128, :], in_=yps[HD:128, :],
                            func=mybir.ActivationFunctionType.Ln,
                        )
                        rsb = npool.tile([128, 512], F32, tag="rsb")
                        nc.scalar.activation(
                            out=rsb[HD:128, :], in_=lg[HD:128, :],
                            func=mybir.ActivationFunctionType.Exp, scale=-1.0,
                        )
                        rln = npool.tile([HD, 512], F32, tag="rln")
                        nc.sync.dma_start(out=rln, in_=rsb[HD:128, :])
                        nc.vector.tensor_mul(YT[h][:, ss], yps[0:HD, :], rln)

                    # projection for this si's four 128-token chunks
                    for sci in range(4 * si, 4 * si + 4):
                        scs = slice(sci * 128, (sci + 1) * 128)
                        po1 = pop.tile([128, 512], F32, tag="po")
                        po2 = pop.tile([128, 512], F32, tag="po")
                        for h in range(HPC):
                            nc.tensor.matmul(
                                po1,
                                lhsT=YT[h][:, scs],
                                rhs=WP[h][:, 0:512],
                                start=(h == 0),
                                stop=(h == HPC - 1),
                            )
                            nc.tensor.matmul(
                                po2[:, 0:256],
                                lhsT=YT[h][:, scs],
                                rhs=WP[h][:, 512:768],
                                start=(h == 0),
                                stop=(h == HPC - 1),
                            )
                        ost = ostp.tile([128, D], F32, tag="ost")
                        nc.scalar.copy(out=ost[:, 0:512], in_=po1)
                        nc.vector.tensor_copy(out=ost[:, 512:768], in_=po2[:, 0:256])
                        nc.sync.dma_start(out=out_d[scs, :], in_=ost)

    return nc


_PROGRAM = None


def _get_program() -> bass.Bass:
    global _PROGRAM
    if _PROGRAM is None:
        _PROGRAM = build_program()
        _split_multi_waits(_PROGRAM)
    return _PROGRAM


def _bf16(arr):
    return np.ascontiguousarray(arr).astype(mybir.dt.np(BF16))


def make_in_maps(x, Wq, Wk, Wv, Wproj):
    x = np.asarray(x, dtype=np.float32)
    Wq = np.asarray(Wq, dtype=np.float32)
    Wk = np.asarray(Wk, dtype=np.float32)
    Wv = np.asarray(Wv, dtype=np.float32)
    Wproj = np.asarray(Wproj, dtype=np.float32)

    half = HD // 2
    j = np.arange(half, dtype=np.float64)
    freq = 1.0 / (10000.0 ** (j / half))
    ang = np.arange(S, dtype=np.float64)[None, :] * freq[:, None]   # [32, S]
    cosT = np.cos(ang).astype(np.float32)
    sinT = np.sin(ang).astype(np.float32)
    csc = np.tile(np.vstack([cosT, cosT]), (2, 1))                  # [128, S]
    css = np.tile(np.vstack([-sinT, sinT]), (2, 1))

    swp = np.zeros((128, 128), dtype=np.float32)
    for blk in range(2):
        for jj in range(half):
            swp[blk * 64 + jj, blk * 64 + half + jj] = 1.0
            swp[blk * 64 + half + jj, blk * 64 + jj] = 1.0

    perm = np.concatenate([np.arange(0, HD, 2), np.arange(1, HD, 2)])

    in_maps = []
    for c in range(NCORES):
        b = c // 4
        hs = [(c % 4) * HPC + i for i in range(HPC)]
        rq = [Wq[h * HD:(h + 1) * HD][perm, :] for h in hs]
        rk = [Wk[h * HD:(h + 1) * HD][perm, :] for h in hs]
        rv = [Wv[h * HD:(h + 1) * HD, :] for h in hs]
        cols = np.concatenate(
            [rq[0], rq[1], rk[0], rk[1], rq[2], rk[2], rv[0], rv[1], rv[2]],
            axis=0,
        )                                                           # [576, D]
        wallm = np.ascontiguousarray(cols.T)                        # [D, 576]
        dims = np.concatenate([np.arange(h * HD, (h + 1) * HD) for h in hs])
        wproj_t = np.ascontiguousarray(Wproj[:, dims].T)            # [192, D]
        in_maps.append({
            "xt": _bf16(x[b].T),
            "wall": _bf16(wallm),
            "wproj": _bf16(wproj_t),
            "csc": _bf16(csc),
            "css": _bf16(css),
            "swp": _bf16(swp),
        })
    return in_maps


def kernel(x, Wq, Wk, Wv, Wproj):
    global LAST_EXEC_NS, LAST_RESULTS
    nc = _get_program()
    in_maps = make_in_maps(x, Wq, Wk, Wv, Wproj)
    trace = os.environ.get("KERNEL_TRACE", "0") == "1"
    res = run_bass_kernel_spmd(nc, in_maps, list(range(NCORES)), trace=trace)
    LAST_EXEC_NS = res.exec_time_ns
    LAST_RESULTS = res
    outs = [np.asarray(r["out"], dtype=np.float32) for r in res.results]
    out = np.empty((2, S, D), dtype=np.float32)
    out[0] = outs[0] + outs[1] + outs[2] + outs[3]
    out[1] = outs[4] + outs[5] + outs[6] + outs[7]
    return out


# revision 10
# speedup vs baseline: 1.3280x; 1.0686x over previous
"""Trainium2 Bass kernel for nn_CausalSelfAttention (erf-kernel attention).

Sharding: 8 cores = 2 batches x 4 core-groups; each core handles one batch
and 3 of the 12 heads.  Each core computes its 3 heads' attention plus its
partial output projection; the host sums the 4 partials per batch.

v2 design (bf16 storage, fp32 PSUM accumulation):
  - Host packs q/k weight rows (rope-permuted: even dims then odd dims) into
    a [768, 384] "wall" (chunks: [q0|q1], [k0|k1], [q2|k2]) plus v rows
    [768, 192].  q/k computed via 3x4x6 matmuls into C tiles; v computed
    DIRECTLY TRANSPOSED per 128-token chunk (lhsT = xT chunk, rhs = WvT)
    so no PE transposes are needed.
  - v lands in VEXT [128, 16*384]: per chunk c, head h the 128 columns
    [v_h (64) | ones (64)].  The ones columns make the AV matmul emit the
    softmax denominator replicated across 64 partitions for free (M=128).
  - RoPE: partner swap via PE permutation matmul, then 3 DVE ops in bf16.
  - Scores per (si, t-chunk): [128 t, 512 s] tiles, causal chunks only,
    PAIRED into [128, 1024] PSUM regions so erf runs once per pair.
    weights = erf(0.125*scores) + 1 (ACT erf, DVE +1), diagonal band
    masked with affine_select on Pool.
  - AV accumulates yps [128, 512]: rows 0:64 = y, rows 64:128 = denom
    (identical).  reciprocal_approx_fast on rows 64:128, small SBUF->SBUF
    DMA realigns the reciprocal rows to 0:64 (compute engines are
    lane-locked; DMA moves partitions freely), one DVE mul normalizes.
  - Projection per 128-token chunk, PSUM-accumulated over 3 heads,
    copied to SBUF (ACT/Pool) and DMA'd out in fp32.
"""

import os
import sys
from contextlib import ExitStack

import numpy as np

for _p in ("/opt/trn_rl_repo",):
    if _p not in sys.path:
        sys.path.insert(0, _p)

import concourse.bass as bass
import concourse.mybir as mybir
from concourse.bass_utils import run_bass_kernel_spmd
from concourse.tile import TileContext

S = 2048          # sequence length per batch
D = 768           # model dim
HD = 64           # head dim
HPC = 3           # heads per core
NCORES = 8
F32 = mybir.dt.float32
BF16 = mybir.dt.bfloat16
NT = S // 512     # 4 free-dim tiles of 512
TC = S // 128     # 16 t-chunks of 128

# CoreSim doesn't implement Erf; dev-only switch to validate logic in sim.
ERF_FUNC_NAME = "Tanh" if os.environ.get("KERNEL_SIM_TANH", "0") == "1" else "Erf"

LAST_EXEC_NS = None
LAST_RESULTS = None


def _split_multi_waits(nc: bass.Bass) -> None:
    """This walrus build rejects instructions carrying more than one sync
    wait (codegen 'Too many sync wait commands', hit by the Tile kernel-tail
    drain).  Hoist all but the last wait of any multi-wait instruction onto
    single-wait Drain instructions inserted just before it on the same
    engine — semantically identical, one wait per instruction."""
    for f in nc.m.functions:
        for b in f.blocks:
            new_insts = []
            changed = False
            for inst in b.instructions:
                si = inst.sync_info
                waits = list(si.on_wait) if si is not None and si.on_wait else []
                if len(waits) > 1:
                    changed = True
                    for n, w in enumerate(waits[:-1]):
                        d = mybir.InstDrain(
                            name=f"{inst.name}-wsplit{n}",
                            engine=inst.engine,
                            ins=[],
                            outs=[],
                            sync_info=mybir.SyncInfo(on_wait=[w], on_update=[]),
                        )
                        new_insts.append(d)
                    si.on_wait = [waits[-1]]
                new_insts.append(inst)
            if changed:
                b.instructions[:] = new_insts


def build_program() -> bass.Bass:
    nc = bass.Bass(target_bir_lowering=False, debug=False)

    x_t = nc.declare_dram_parameter("xt", [D, S], BF16, isOutput=False)
    wall = nc.declare_dram_parameter("wall", [D, 576], BF16, isOutput=False)
    wproj = nc.declare_dram_parameter("wproj", [HPC * HD, D], BF16, isOutput=False)
    csc = nc.declare_dram_parameter("csc", [128, S], BF16, isOutput=False)
    css = nc.declare_dram_parameter("css", [128, S], BF16, isOutput=False)
    swp = nc.declare_dram_parameter("swp", [128, 128], BF16, isOutput=False)
    out_d = nc.declare_dram_parameter("out", [S, D], F32, isOutput=True)

    erf_func = getattr(mybir.ActivationFunctionType, ERF_FUNC_NAME)

    with TileContext(nc) as tc:
        with ExitStack() as ctx:
            const = ctx.enter_context(tc.tile_pool(name="const", bufs=1))
            tpool = ctx.enter_context(tc.tile_pool(name="tpool", bufs=3))
            wtp = ctx.enter_context(tc.tile_pool(name="wtp", bufs=5))
            nrm = ctx.enter_context(tc.tile_pool(name="nrm", bufs=2))
            ostp = ctx.enter_context(tc.tile_pool(name="ostp", bufs=3))

            # ---- inputs ----
            WA = []
            for kc in range(6):
                t = const.tile([128, 576], BF16, tag=f"wa{kc}", name=f"wa{kc}")
                nc.sync.dma_start(out=t, in_=wall[kc * 128:(kc + 1) * 128, :])
                WA.append(t)
            XT = []
            for kc in range(6):
                t = const.tile([128, S], BF16, tag=f"xt{kc}", name=f"xt{kc}")
                XT.append(t)
            # nt-major DMA order so the first QKV matmuls start early
            for ntb in range(NT):
                ns = slice(ntb * 512, (ntb + 1) * 512)
                for kc in range(6):
                    nc.sync.dma_start(out=XT[kc][:, ns], in_=x_t[kc * 128:(kc + 1) * 128, ns])
            CSC = const.tile([128, S], BF16, tag="csc")
            nc.sync.dma_start(out=CSC, in_=csc[:, :])
            CSS = const.tile([128, S], BF16, tag="css")
            nc.sync.dma_start(out=CSS, in_=css[:, :])
            SWP = const.tile([128, 128], BF16, tag="swp")
            nc.sync.dma_start(out=SWP, in_=swp[:, :])
            WP = []
            for h in range(HPC):
                t = const.tile([HD, D], BF16, tag=f"wp{h}", name=f"wp{h}")
                nc.sync.dma_start(out=t, in_=wproj[h * HD:(h + 1) * HD, :])
                WP.append(t)

            # VEXT: per chunk c, head h: cols c*384 + h*128 + [v(64) | ones(64)]
            VEXT = const.tile([128, TC * 384], BF16, tag="vext")
            v4 = VEXT.rearrange("p (c h x) -> p c h x", c=TC, h=HPC, x=128)
            for h in range(HPC):
                nc.gpsimd.memset(v4[:, :, h, HD:128], 1.0)

            # persistent q/k tiles (raw then roped)
            C = [const.tile([128, S], BF16, tag=f"c{m}", name=f"c{m}") for m in range(3)]
            R = [const.tile([128, S], BF16, tag=f"r{m}", name=f"r{m}") for m in range(3)]
            # k2 relocated to base partition 0 (matmul requires lhsT/rhs at
            # the same base partition; q2 is at rows 0:64 of R[2])
            K2 = const.tile([HD, S], BF16, tag="k2")
            YT = [const.tile([HD, S], BF16, tag=f"yt{h}", name=f"yt{h}") for h in range(HPC)]

            # ---- phase 1: QKV + RoPE (own PSUM scope) ----
            with tc.tile_pool(name="qkps", bufs=2, space="PSUM") as qkps, \
                 tc.tile_pool(name="vtps", bufs=2, space="PSUM") as vtps, \
                 tc.tile_pool(name="swps", bufs=2, space="PSUM") as swps:
                # q/k wall matmuls: C[m] rows = [q0|q1], [k0|k1], [q2|k2]
                for m in range(3):
                    for ntb in range(NT):
                        ns = slice(ntb * 512, (ntb + 1) * 512)
                        ps = qkps.tile([128, 512], F32, tag="qk")
                        for kc in range(6):
                            nc.tensor.matmul(
                                ps,
                                lhsT=WA[kc][:, m * 128:(m + 1) * 128],
                                rhs=XT[kc][:, ns],
                                start=(kc == 0),
                                stop=(kc == 5),
                            )
                        nc.vector.tensor_copy(out=C[m][:, ns], in_=ps)

                # v directly transposed: per t-chunk, [128 t, 192 d]
                for tcb in range(TC):
                    ts = slice(tcb * 128, (tcb + 1) * 128)
                    ps = vtps.tile([128, 512], F32, tag="vt")
                    for kc in range(6):
                        nc.tensor.matmul(
                            ps[:, 0:HPC * HD],
                            lhsT=XT[kc][:, ts],
                            rhs=WA[kc][:, 384:576],
                            start=(kc == 0),
                            stop=(kc == 5),
                        )
                    src = ps[:, 0:HPC * HD].rearrange("p (h x) -> p h x", h=HPC, x=HD)
                    nc.vector.tensor_copy(out=v4[:, tcb, :, 0:HD], in_=src)

                # RoPE: R[m] = C[m]*csc + swap(C[m])*css
                for ntb in range(NT):
                    ns = slice(ntb * 512, (ntb + 1) * 512)
                    for m in range(3):
                        sw = swps.tile([128, 512], F32, tag="sw")
                        nc.tensor.matmul(
                            sw, lhsT=SWP, rhs=C[m][:, ns], start=True, stop=True
                        )
                        t1 = tpool.tile([128, 512], BF16, tag="t1")
                        t2 = tpool.tile([128, 512], BF16, tag="t2")
                        nc.vector.tensor_mul(t1, C[m][:, ns], CSC[:, ns])
                        nc.vector.tensor_mul(t2, sw, CSS[:, ns])
                        nc.vector.tensor_add(R[m][:, ns], t1, t2)
                        if m == 2:
                            nc.sync.dma_start(out=K2[:, ns], in_=R[2][HD:128, ns])

            QS = [R[0][0:HD, :], R[0][HD:128, :], R[2][0:HD, :]]
            KS = [R[1][0:HD, :], R[1][HD:128, :], K2[:, :]]

            # unnormalized y (rows 0:64) + denom (rows 64:128), fp32
            YU = [const.tile([128, S], F32, tag=f"yu{h}", name=f"yu{h}") for h in range(HPC)]

            # ---- phase 2: attention (software-pipelined pair stream) ----
            # Pair tasks across (si, h); AV matmuls lag LAG pairs behind the
            # score matmuls so the PE never waits on the erf/add/select chain.
            LAG = 3
            tasks = []
            for si in range(NT):
                for h in range(HPC):
                    npair = 2 * (si + 1)
                    for p in range(npair):
                        tasks.append((si, h, p, npair))

            with tc.tile_pool(name="scps", bufs=3, space="PSUM") as scps, \
                 tc.tile_pool(name="ypsp", bufs=2, space="PSUM") as ypsp:
                ypsmap = {}
                pend = []

                def emit_av(task, wt):
                    si, h, p, npair = task
                    ss = slice(si * 512, (si + 1) * 512)
                    key = (si, h)
                    if key not in ypsmap:
                        ypsmap[key] = ypsp.tile([128, 512], F32, tag="yps", name=f"yps{si}_{h}")
                    yps = ypsmap[key]
                    for half in range(2):
                        tcb = 2 * p + half
                        nc.tensor.matmul(
                            yps,
                            lhsT=VEXT[:, tcb * 384 + h * 128:tcb * 384 + (h + 1) * 128],
                            rhs=wt[:, half * 512:(half + 1) * 512],
                            start=(tcb == 0),
                            stop=(tcb == 2 * npair - 1),
                        )
                    if p == npair - 1:
                        nc.vector.tensor_copy(out=YU[h][:, ss], in_=yps)
                        del ypsmap[key]

                for task in tasks:
                    si, h, p, npair = task
                    ss = slice(si * 512, (si + 1) * 512)
                    sc = scps.tile([128, 1024], F32, tag="sc")
                    for half in range(2):
                        tcb = 2 * p + half
                        nc.tensor.matmul(
                            sc[:, half * 512:(half + 1) * 512],
                            lhsT=KS[h][:, tcb * 128:(tcb + 1) * 128],
                            rhs=QS[h][:, ss],
                            start=True,
                            stop=True,
                        )
                    wt = wtp.tile([128, 1024], BF16, tag="wt")
                    nc.scalar.activation(out=wt, in_=sc, func=erf_func, scale=0.125)
                    nc.vector.tensor_scalar_add(wt, wt, 1.0)
                    if p >= 2 * si:  # diagonal band pair
                        for half in range(2):
                            tcb = 2 * p + half
                            nc.gpsimd.affine_select(
                                out=wt[:, half * 512:(half + 1) * 512],
                                in_=wt[:, half * 512:(half + 1) * 512],
                                compare_op=mybir.AluOpType.is_ge,
                                fill=0.0,
                                base=si * 512 - tcb * 128,
                                channel_multiplier=-1,
                                pattern=[[1, 512]],
                            )
                    pend.append((task, wt))
                    if len(pend) > LAG:
                        emit_av(*pend.pop(0))
                while pend:
                    emit_av(*pend.pop(0))

            # ---- phase 3: normalize (2 table loads total) + projection ----
            with tc.tile_pool(name="pop", bufs=4, space="PSUM") as pop:
                LG = [nrm.tile([128, S], F32, tag="lg", name=f"lg{h}", bufs=3) for h in range(HPC)]
                for h in range(HPC):
                    nc.scalar.activation(
                        out=LG[h][HD:128, :], in_=YU[h][HD:128, :],
                        func=mybir.ActivationFunctionType.Ln,
                    )
                for h in range(HPC):
                    rs = nrm.tile([128, S], F32, tag="rs", name=f"rs{h}")
                    nc.scalar.activation(
                        out=rs[HD:128, :], in_=LG[h][HD:128, :],
                        func=mybir.ActivationFunctionType.Exp, scale=-1.0,
                    )
                    rl = nrm.tile([HD, S], F32, tag="rl", name=f"rl{h}")
                    nc.sync.dma_start(out=rl, in_=rs[HD:128, :])
                    nc.vector.tensor_mul(YT[h], YU[h][0:HD, :], rl)

                for sci in range(TC):
                    scs = slice(sci * 128, (sci + 1) * 128)
                    po1 = pop.tile([128, 512], F32, tag="po")
                    po2 = pop.tile([128, 512], F32, tag="po")
                    for h in range(HPC):
                        nc.tensor.matmul(
                            po1,
                            lhsT=YT[h][:, scs],
                            rhs=WP[h][:, 0:512],
                            start=(h == 0),
                            stop=(h == HPC - 1),
                        )
                        nc.tensor.matmul(
                            po2[:, 0:256],
                            lhsT=YT[h][:, scs],
                            rhs=WP[h][:, 512:768],
                            start=(h == 0),
                            stop=(h == HPC - 1),
                        )
                    ost = ostp.tile([128, D], F32, tag="ost")
                    if sci % 2 == 0:
                        nc.scalar.copy(out=ost[:, 0:512], in_=po1)
                        nc.vector.tensor_copy(out=ost[:, 512:768], in_=po2[:, 0:256])
                    else:
                        nc.vector.tensor_copy(out=ost[:, 0:512], in_=po1)
                        nc.scalar.copy(out=ost[:, 512:768], in_=po2[:, 0:256])
                    nc.sync.dma_start(out=out_d[scs, :], in_=ost)

    return nc


_PROGRAM = None


def _get_program() -> bass.Bass:
    global _PROGRAM
    if _PROGRAM is None:
        _PROGRAM = build_program()
        _split_multi_waits(_PROGRAM)
    return _PROGRAM


def _bf16(arr):
    return np.ascontiguousarray(arr).astype(mybir.dt.np(BF16))


def make_in_maps(x, Wq, Wk, Wv, Wproj):
    x = np.asarray(x, dtype=np.float32)
    Wq = np.asarray(Wq, dtype=np.float32)
    Wk = np.asarray(Wk, dtype=np.float32)
    Wv = np.asarray(Wv, dtype=np.float32)
    Wproj = np.asarray(Wproj, dtype=np.float32)

    half = HD // 2
    j = np.arange(half, dtype=np.float64)
    freq = 1.0 / (10000.0 ** (j / half))
    ang = np.arange(S, dtype=np.float64)[None, :] * freq[:, None]   # [32, S]
    cosT = np.cos(ang).astype(np.float32)
    sinT = np.sin(ang).astype(np.float32)
    csc = np.tile(np.vstack([cosT, cosT]), (2, 1))                  # [128, S]
    css = np.tile(np.vstack([-sinT, sinT]), (2, 1))

    swp = np.zeros((128, 128), dtype=np.float32)
    for blk in range(2):
        for jj in range(half):
            swp[blk * 64 + jj, blk * 64 + half + jj] = 1.0
            swp[blk * 64 + half + jj, blk * 64 + jj] = 1.0

    perm = np.concatenate([np.arange(0, HD, 2), np.arange(1, HD, 2)])

    in_maps = []
    for c in range(NCORES):
        b = c // 4
        hs = [(c % 4) * HPC + i for i in range(HPC)]
        rq = [Wq[h * HD:(h + 1) * HD][perm, :] for h in hs]
        rk = [Wk[h * HD:(h + 1) * HD][perm, :] for h in hs]
        rv = [Wv[h * HD:(h + 1) * HD, :] for h in hs]
        cols = np.concatenate(
            [rq[0], rq[1], rk[0], rk[1], rq[2], rk[2], rv[0], rv[1], rv[2]],
            axis=0,
        )                                                           # [576, D]
        wallm = np.ascontiguousarray(cols.T)                        # [D, 576]
        dims = np.concatenate([np.arange(h * HD, (h + 1) * HD) for h in hs])
        wproj_t = np.ascontiguousarray(Wproj[:, dims].T)            # [192, D]
        in_maps.append({
            "xt": _bf16(x[b].T),
            "wall": _bf16(wallm),
            "wproj": _bf16(wproj_t),
            "csc": _bf16(csc),
            "css": _bf16(css),
            "swp": _bf16(swp),
        })
    return in_maps


def kernel(x, Wq, Wk, Wv, Wproj):
    global LAST_EXEC_NS, LAST_RESULTS
    nc = _get_program()
    in_maps = make_in_maps(x, Wq, Wk, Wv, Wproj)
    trace = os.environ.get("KERNEL_TRACE", "0") == "1"
    res = run_bass_kernel_spmd(nc, in_maps, list(range(NCORES)), trace=trace)
    LAST_EXEC_NS = res.exec_time_ns
    LAST_RESULTS = res
    outs = [np.asarray(r["out"], dtype=np.float32) for r in res.results]
    out = np.empty((2, S, D), dtype=np.float32)
    out[0] = outs[0] + outs[1] + outs[2] + outs[3]
    out[1] = outs[4] + outs[5] + outs[6] + outs[7]
    return out


# revision 11
# speedup vs baseline: 1.5889x; 1.1965x over previous
"""Trainium2 Bass kernel for nn_CausalSelfAttention (erf-kernel attention).

Sharding: 8 cores = 2 batches x 4 core-groups; each core handles one batch
and 3 of the 12 heads.  Each core computes its 3 heads' attention plus its
partial output projection; the host sums the 4 partials per batch.

v4 design (bf16 storage, fp32 PSUM accumulation):
  - ONE merged instruction stream keeps the PE continuously busy so the
    DVFS p-state ramps to 2.4 GHz: per nt-block emit [QKV(nt), rope(nt),
    vT chunks, attention(si=nt)]; all through one rotating PSUM pool.
  - Host packs q/k weight rows (rope-permuted: even dims then odd dims)
    into wall chunks [q0|q1], [k0|k1], [q2|k2] plus v rows.  v is computed
    directly transposed (lhsT = xT chunk) - no PE transposes.
  - VEXT [128, 16*384]: per chunk c, head h: 128 cols = [v|ones] (h0,h2)
    or [ones|v] (h1).  The ones columns make the AV matmul emit the
    denominator replicated across 64 partitions for free (M=128); h1's
    inverted layout puts its y rows at partitions 64:128 so the
    normalize-mul can write YT01[64:128] lane-aligned, enabling K=128
    head-paired projection matmuls.
  - Scores per (si, t-chunk): [128 t, 512 s], causal chunks only, PAIRED
    into [128,1024] PSUM so erf runs once per pair; band chunks compute
    only the valid column range (causal column reduction).
  - weights = erf(0.125*scores) + 1: ACT erf (the only table resident
    during the stream), DVE +1, Pool affine_select on band chunks.  AV
    matmuls lag LAG pairs behind scores (software pipelining).
  - Unnormalized y+denom copied to SBUF per (h,si); tail phase does
    1/d = exp(-ln(d)) on ACT (2 table loads total), a small SBUF->SBUF
    DMA realigns reciprocal rows across partitions, one DVE mul per head,
    then the head-paired projection, PSUM->SBUF copies, DMA out in fp32.
"""

import os
import sys
from contextlib import ExitStack

import numpy as np

for _p in ("/opt/trn_rl_repo",):
    if _p not in sys.path:
        sys.path.insert(0, _p)

import concourse.bass as bass
import concourse.mybir as mybir
from concourse.bass_utils import run_bass_kernel_spmd
from concourse.tile import TileContext

S = 2048          # sequence length per batch
D = 768           # model dim
HD = 64           # head dim
HPC = 3           # heads per core
NCORES = 8
F32 = mybir.dt.float32
BF16 = mybir.dt.bfloat16
NT = S // 512     # 4 free-dim tiles of 512
TC = S // 128     # 16 t-chunks of 128

# CoreSim doesn't implement Erf; dev-only switch to validate logic in sim.
ERF_FUNC_NAME = "Tanh" if os.environ.get("KERNEL_SIM_TANH", "0") == "1" else "Erf"

LAST_EXEC_NS = None
LAST_RESULTS = None


def _split_multi_waits(nc: bass.Bass) -> None:
    """This walrus build rejects instructions carrying more than one sync
    wait (codegen 'Too many sync wait commands', hit by the Tile kernel-tail
    drain).  Hoist all but the last wait of any multi-wait instruction onto
    single-wait Drain instructions inserted just before it on the same
    engine — semantically identical, one wait per instruction."""
    for f in nc.m.functions:
        for b in f.blocks:
            new_insts = []
            changed = False
            for inst in b.instructions:
                si = inst.sync_info
                waits = list(si.on_wait) if si is not None and si.on_wait else []
                if len(waits) > 1:
                    changed = True
                    for n, w in enumerate(waits[:-1]):
                        d = mybir.InstDrain(
                            name=f"{inst.name}-wsplit{n}",
                            engine=inst.engine,
                            ins=[],
                            outs=[],
                            sync_info=mybir.SyncInfo(on_wait=[w], on_update=[]),
                        )
                        new_insts.append(d)
                    si.on_wait = [waits[-1]]
                new_insts.append(inst)
            if changed:
                b.instructions[:] = new_insts


def build_program() -> bass.Bass:
    nc = bass.Bass(target_bir_lowering=False, debug=False)

    x_t = nc.declare_dram_parameter("xt", [D, S], BF16, isOutput=False)
    wall = nc.declare_dram_parameter("wall", [D, 576], BF16, isOutput=False)
    wproj = nc.declare_dram_parameter("wproj", [HPC * HD, D], BF16, isOutput=False)
    csc = nc.declare_dram_parameter("csc", [128, S], BF16, isOutput=False)
    css = nc.declare_dram_parameter("css", [128, S], BF16, isOutput=False)
    swp = nc.declare_dram_parameter("swp", [128, 128], BF16, isOutput=False)
    out_d = nc.declare_dram_parameter("out", [S, D], F32, isOutput=True)

    erf_func = getattr(mybir.ActivationFunctionType, ERF_FUNC_NAME)
    # per-head row split of the AV output: YROW = y rows, DROW = denom rows.
    YROW = [slice(0, HD), slice(HD, 128), slice(0, HD)]
    DROW = [slice(HD, 128), slice(0, HD), slice(HD, 128)]
    OOFF = [HD, 0, HD]     # ones block offset within the head's 128 cols

    with TileContext(nc) as tc:
        with ExitStack() as ctx:
            const = ctx.enter_context(tc.tile_pool(name="const", bufs=1))
            tpool = ctx.enter_context(tc.tile_pool(name="tpool", bufs=3))
            wtp = ctx.enter_context(tc.tile_pool(name="wtp", bufs=5))
            nrm = ctx.enter_context(tc.tile_pool(name="nrm", bufs=2))
            ostp = ctx.enter_context(tc.tile_pool(name="ostp", bufs=3))

            # ---- input DMAs (ordered so compute can start early) ----
            WA = []
            for kc in range(6):
                t = const.tile([128, 576], BF16, tag=f"wa{kc}", name=f"wa{kc}")
                nc.sync.dma_start(out=t, in_=wall[kc * 128:(kc + 1) * 128, :])
                WA.append(t)
            XT = [const.tile([128, S], BF16, tag=f"xt{kc}", name=f"xt{kc}")
                  for kc in range(6)]
            ns0 = slice(0, 512)
            for kc in range(6):
                nc.sync.dma_start(out=XT[kc][:, ns0], in_=x_t[kc * 128:(kc + 1) * 128, ns0])
            SWP = const.tile([128, 128], BF16, tag="swp")
            nc.sync.dma_start(out=SWP, in_=swp[:, :])
            CSC = const.tile([128, S], BF16, tag="csc")
            nc.sync.dma_start(out=CSC, in_=csc[:, :])
            CSS = const.tile([128, S], BF16, tag="css")
            nc.sync.dma_start(out=CSS, in_=css[:, :])
            for ntb in range(1, NT):
                ns = slice(ntb * 512, (ntb + 1) * 512)
                for kc in range(6):
                    nc.sync.dma_start(out=XT[kc][:, ns], in_=x_t[kc * 128:(kc + 1) * 128, ns])
            WPP = const.tile([128, D], BF16, tag="wpp")
            nc.sync.dma_start(out=WPP, in_=wproj[0:128, :])
            WP2 = const.tile([HD, D], BF16, tag="wp2")
            nc.sync.dma_start(out=WP2, in_=wproj[128:192, :])

            # VEXT: per chunk c, head h: 128 cols = [v|ones] (h0,h2), [ones|v] (h1)
            VEXT = const.tile([128, TC * 384], BF16, tag="vext")
            v4 = VEXT.rearrange("p (c h x) -> p c h x", c=TC, h=HPC, x=128)
            for h in range(HPC):
                nc.gpsimd.memset(v4[:, :, h, OOFF[h]:OOFF[h] + HD], 1.0)

            C = [const.tile([128, S], BF16, tag=f"c{m}", name=f"c{m}") for m in range(3)]
            R = [const.tile([128, S], BF16, tag=f"r{m}", name=f"r{m}") for m in range(3)]
            # k2 relocated to base partition 0 (matmul requires lhsT/rhs at
            # the same base partition; q2 is at rows 0:64 of R[2])
            K2 = const.tile([HD, S], BF16, tag="k2")
            # heads 0 (rows 0:64) and 1 (rows 64:128) share YT01 so the
            # projection can pair them into K=128 matmuls
            YT01 = const.tile([128, S], BF16, tag="yt01")
            YT2 = const.tile([HD, S], BF16, tag="yt2")
            YU = [const.tile([128, S], F32, tag=f"yu{h}", name=f"yu{h}") for h in range(HPC)]

            QS = [R[0][0:HD, :], R[0][HD:128, :], R[2][0:HD, :]]
            KS = [R[1][0:HD, :], R[1][HD:128, :], K2[:, :]]

            # ---- merged stream: per nt-block [QKV(nt), rope(nt), vT chunks,
            # attention(si=nt)]; AV matmuls lag LAG pairs behind scores ----
            LAG = 3
            with tc.tile_pool(name="mps", bufs=3, space="PSUM") as mps, \
                 tc.tile_pool(name="ypsp", bufs=2, space="PSUM") as ypsp:
                ypsmap = {}
                pend = []

                def emit_av(task, wt):
                    si, h, p, npair = task
                    key = (si, h)
                    if key not in ypsmap:
                        ypsmap[key] = ypsp.tile(
                            [128, 512], F32, tag="yps", name=f"yps{si}_{h}")
                    yps = ypsmap[key]
                    for half in range(2):
                        tcb = 2 * p + half
                        lo = max(0, tcb - 4 * si) * 128
                        nc.tensor.matmul(
                            yps[:, lo:512],
                            lhsT=VEXT[:, tcb * 384 + h * 128:tcb * 384 + (h + 1) * 128],
                            rhs=wt[:, half * 512 + lo:(half + 1) * 512],
                            start=(tcb == 0),
                            stop=(tcb == 2 * npair - 1),
                            skip_group_check=True,
                        )
                    if p == npair - 1:
                        ss = slice(si * 512, (si + 1) * 512)
                        nc.vector.tensor_copy(out=YU[h][:, ss], in_=yps)
                        del ypsmap[key]

                for ntb in range(NT):
                    ns = slice(ntb * 512, (ntb + 1) * 512)
                    # q/k wall matmuls for this nt
                    for m in range(3):
                        ps = mps.tile([128, 1024], F32, tag="m", name=f"qk{m}_{ntb}")
                        for kc in range(6):
                            nc.tensor.matmul(
                                ps[:, 0:512],
                                lhsT=WA[kc][:, m * 128:(m + 1) * 128],
                                rhs=XT[kc][:, ns],
                                start=(kc == 0),
                                stop=(kc == 5),
                            )
                        nc.scalar.copy(out=C[m][:, ns], in_=ps[:, 0:512])
                    # rope for this nt
                    for m in range(3):
                        sw = mps.tile([128, 1024], F32, tag="m", name=f"sw{m}_{ntb}")
                        nc.tensor.matmul(
                            sw[:, 0:512], lhsT=SWP, rhs=C[m][:, ns],
                            start=True, stop=True,
                        )
                        t1 = tpool.tile([128, 512], BF16, tag="t1")
                        t2 = tpool.tile([128, 512], BF16, tag="t2")
                        nc.vector.tensor_mul(t1, C[m][:, ns], CSC[:, ns])
                        nc.vector.tensor_mul(t2, sw[:, 0:512], CSS[:, ns])
                        nc.vector.tensor_add(R[m][:, ns], t1, t2)
                        if m == 2:
                            nc.sync.dma_start(out=K2[:, ns], in_=R[2][HD:128, ns])
                    # vT chunks for this nt
                    for tcb in range(4 * ntb, 4 * ntb + 4):
                        ts = slice(tcb * 128, (tcb + 1) * 128)
                        ps = mps.tile([128, 1024], F32, tag="m", name=f"vt{tcb}")
                        for kc in range(6):
                            nc.tensor.matmul(
                                ps[:, 0:HPC * HD],
                                lhsT=XT[kc][:, ts],
                                rhs=WA[kc][:, 384:576],
                                start=(kc == 0),
                                stop=(kc == 5),
                            )
                        base = tcb * 384
                        # v0 -> cols [0:64); v1,v2 -> contiguous cols [192:320)
                        nc.scalar.copy(out=VEXT[:, base:base + HD], in_=ps[:, 0:HD])
                        nc.scalar.copy(
                            out=VEXT[:, base + 192:base + 320], in_=ps[:, HD:3 * HD])
                    # attention for si = ntb
                    si = ntb
                    ss = slice(si * 512, (si + 1) * 512)
                    npair = 2 * (si + 1)
                    for h in range(HPC):
                        for p in range(npair):
                            sc = mps.tile([128, 1024], F32, tag="m",
                                          name=f"sc{si}_{h}_{p}")
                            for half in range(2):
                                tcb = 2 * p + half
                                lo = max(0, tcb - 4 * si) * 128
                                nc.tensor.matmul(
                                    sc[:, half * 512 + lo:(half + 1) * 512],
                                    lhsT=KS[h][:, tcb * 128:(tcb + 1) * 128],
                                    rhs=QS[h][:, si * 512 + lo:(si + 1) * 512],
                                    start=True,
                                    stop=True,
                                )
                            wt = wtp.tile([128, 1024], BF16, tag="wt")
                            if p >= 2 * si:
                                # band pair: erf only the valid column ranges
                                for half in range(2):
                                    tcb = 2 * p + half
                                    lo = max(0, tcb - 4 * si) * 128
                                    nc.scalar.activation(
                                        out=wt[:, half * 512 + lo:(half + 1) * 512],
                                        in_=sc[:, half * 512 + lo:(half + 1) * 512],
                                        func=erf_func, scale=0.125,
                                    )
                            else:
                                nc.scalar.activation(
                                    out=wt, in_=sc, func=erf_func, scale=0.125)
                            nc.vector.tensor_scalar_add(wt, wt, 1.0)
                            if p >= 2 * si:  # diagonal band: causal mask
                                for half in range(2):
                                    tcb = 2 * p + half
                                    lo = max(0, tcb - 4 * si) * 128
                                    nc.gpsimd.affine_select(
                                        out=wt[:, half * 512 + lo:(half + 1) * 512],
                                        in_=wt[:, half * 512 + lo:(half + 1) * 512],
                                        compare_op=mybir.AluOpType.is_ge,
                                        fill=0.0,
                                        base=0,
                                        channel_multiplier=-1,
                                        pattern=[[1, 512 - lo]],
                                    )
                            pend.append(((si, h, p, npair), wt))
                            if len(pend) > LAG:
                                emit_av(*pend.pop(0))
                while pend:
                    emit_av(*pend.pop(0))

            # ---- tail: normalize (2 table loads total) + projection ----
            with tc.tile_pool(name="pop", bufs=4, space="PSUM") as pop:
                LG = [nrm.tile([128, S], F32, tag="lg", name=f"lg{h}", bufs=3)
                      for h in range(HPC)]
                for h in range(HPC):
                    nc.scalar.activation(
                        out=LG[h][DROW[h], :], in_=YU[h][DROW[h], :],
                        func=mybir.ActivationFunctionType.Ln,
                    )
                for h in range(HPC):
                    rs = nrm.tile([128, S], F32, tag="rs", name=f"rs{h}")
                    nc.scalar.activation(
                        out=rs[DROW[h], :], in_=LG[h][DROW[h], :],
                        func=mybir.ActivationFunctionType.Exp, scale=-1.0,
                    )
                    rl = nrm.tile([128, S], F32, tag="rl", name=f"rl{h}")
                    nc.sync.dma_start(out=rl[YROW[h], :], in_=rs[DROW[h], :])
                    ydst = YT01[YROW[h], :] if h < 2 else YT2[:, :]
                    nc.vector.tensor_mul(ydst, YU[h][YROW[h], :], rl[YROW[h], :])

                for sci in range(TC):
                    scs = slice(sci * 128, (sci + 1) * 128)
                    po1 = pop.tile([128, 512], F32, tag="po")
                    po2 = pop.tile([128, 512], F32, tag="po")
                    nc.tensor.matmul(
                        po1, lhsT=YT01[:, scs], rhs=WPP[:, 0:512],
                        start=True, stop=False,
                    )
                    nc.tensor.matmul(
                        po1, lhsT=YT2[:, scs], rhs=WP2[:, 0:512],
                        start=False, stop=True,
                    )
                    nc.tensor.matmul(
                        po2[:, 0:256], lhsT=YT01[:, scs], rhs=WPP[:, 512:768],
                        start=True, stop=False,
                    )
                    nc.tensor.matmul(
                        po2[:, 0:256], lhsT=YT2[:, scs], rhs=WP2[:, 512:768],
                        start=False, stop=True,
                    )
                    ost = ostp.tile([128, D], F32, tag="ost")
                    if sci % 2 == 0:
                        nc.scalar.copy(out=ost[:, 0:512], in_=po1)
                        nc.vector.tensor_copy(out=ost[:, 512:768], in_=po2[:, 0:256])
                    else:
                        nc.vector.tensor_copy(out=ost[:, 0:512], in_=po1)
                        nc.scalar.copy(out=ost[:, 512:768], in_=po2[:, 0:256])
                    nc.sync.dma_start(out=out_d[scs, :], in_=ost)

    return nc


_PROGRAM = None


def _get_program() -> bass.Bass:
    global _PROGRAM
    if _PROGRAM is None:
        _PROGRAM = build_program()
        _split_multi_waits(_PROGRAM)
    return _PROGRAM


def _bf16(arr):
    return np.ascontiguousarray(arr).astype(mybir.dt.np(BF16))


def make_in_maps(x, Wq, Wk, Wv, Wproj):
    x = np.asarray(x, dtype=np.float32)
    Wq = np.asarray(Wq, dtype=np.float32)
    Wk = np.asarray(Wk, dtype=np.float32)
    Wv = np.asarray(Wv, dtype=np.float32)
    Wproj = np.asarray(Wproj, dtype=np.float32)

    half = HD // 2
    j = np.arange(half, dtype=np.float64)
    freq = 1.0 / (10000.0 ** (j / half))
    ang = np.arange(S, dtype=np.float64)[None, :] * freq[:, None]   # [32, S]
    cosT = np.cos(ang).astype(np.float32)
    sinT = np.sin(ang).astype(np.float32)
    csc = np.tile(np.vstack([cosT, cosT]), (2, 1))                  # [128, S]
    css = np.tile(np.vstack([-sinT, sinT]), (2, 1))

    swp = np.zeros((128, 128), dtype=np.float32)
    for blk in range(2):
        for jj in range(half):
            swp[blk * 64 + jj, blk * 64 + half + jj] = 1.0
            swp[blk * 64 + half + jj, blk * 64 + jj] = 1.0

    perm = np.concatenate([np.arange(0, HD, 2), np.arange(1, HD, 2)])

    in_maps = []
    for c in range(NCORES):
        b = c // 4
        hs = [(c % 4) * HPC + i for i in range(HPC)]
        rq = [Wq[h * HD:(h + 1) * HD][perm, :] for h in hs]
        rk = [Wk[h * HD:(h + 1) * HD][perm, :] for h in hs]
        rv = [Wv[h * HD:(h + 1) * HD, :] for h in hs]
        cols = np.concatenate(
            [rq[0], rq[1], rk[0], rk[1], rq[2], rk[2], rv[0], rv[1], rv[2]],
            axis=0,
        )                                                           # [576, D]
        wallm = np.ascontiguousarray(cols.T)                        # [D, 576]
        dims = np.concatenate([np.arange(h * HD, (h + 1) * HD) for h in hs])
        wproj_t = np.ascontiguousarray(Wproj[:, dims].T)            # [192, D]
        in_maps.append({
            "xt": _bf16(x[b].T),
            "wall": _bf16(wallm),
            "wproj": _bf16(wproj_t),
            "csc": _bf16(csc),
            "css": _bf16(css),
            "swp": _bf16(swp),
        })
    return in_maps


def kernel(x, Wq, Wk, Wv, Wproj):
    global LAST_EXEC_NS, LAST_RESULTS
    nc = _get_program()
    in_maps = make_in_maps(x, Wq, Wk, Wv, Wproj)
    trace = os.environ.get("KERNEL_TRACE", "0") == "1"
    res = run_bass_kernel_spmd(nc, in_maps, list(range(NCORES)), trace=trace)
    LAST_EXEC_NS = res.exec_time_ns
    LAST_RESULTS = res
    outs = [np.asarray(r["out"], dtype=np.float32) for r in res.results]
    out = np.empty((2, S, D), dtype=np.float32)
    out[0] = outs[0] + outs[1] + outs[2] + outs[3]
    out[1] = outs[4] + outs[5] + outs[6] + outs[7]
    return out


# revision 14
# speedup vs baseline: 1.7003x; 1.0701x over previous
"""Trainium2 Bass kernel for nn_CausalSelfAttention (erf-kernel attention).

Sharding: 8 cores = 2 batches x 4 core-groups; each core handles one batch
and 3 of the 12 heads.  Each core computes its 3 heads' attention plus its
partial output projection; the host sums the 4 partials per batch.

v4 design (bf16 storage, fp32 PSUM accumulation):
  - ONE merged instruction stream keeps the PE continuously busy so the
    DVFS p-state ramps to 2.4 GHz: per nt-block emit [QKV(nt), rope(nt),
    vT chunks, attention(si=nt)]; all through one rotating PSUM pool.
  - Host packs q/k weight rows (rope-permuted: even dims then odd dims)
    into wall chunks [q0|q1], [k0|k1], [q2|k2] plus v rows.  v is computed
    directly transposed (lhsT = xT chunk) - no PE transposes.
  - VEXT [128, 16*384]: per chunk c, head h: 128 cols = [v|ones] (h0,h2)
    or [ones|v] (h1).  The ones columns make the AV matmul emit the
    denominator replicated across 64 partitions for free (M=128); h1's
    inverted layout puts its y rows at partitions 64:128 so the
    normalize-mul can write YT01[64:128] lane-aligned, enabling K=128
    head-paired projection matmuls.
  - Scores per (si, t-chunk): [128 t, 512 s], causal chunks only, PAIRED
    into [128,1024] PSUM so erf runs once per pair; band chunks compute
    only the valid column range (causal column reduction).
  - weights = erf(0.125*scores) + 1: ACT erf (the only table resident
    during the stream), DVE +1, Pool affine_select on band chunks.  AV
    matmuls lag LAG pairs behind scores (software pipelining).
  - Unnormalized y+denom copied to SBUF per (h,si); tail phase does
    1/d = exp(-ln(d)) on ACT (2 table loads total), a small SBUF->SBUF
    DMA realigns reciprocal rows across partitions, one DVE mul per head,
    then the head-paired projection, PSUM->SBUF copies, DMA out in fp32.
"""

import os
import sys
from contextlib import ExitStack

import numpy as np

for _p in ("/opt/trn_rl_repo",):
    if _p not in sys.path:
        sys.path.insert(0, _p)

import concourse.bass as bass
import concourse.mybir as mybir
from concourse.bass_utils import run_bass_kernel_spmd
from concourse.tile import TileContext

S = 2048          # sequence length per batch
D = 768           # model dim
HD = 64           # head dim
HPC = 3           # heads per core
NCORES = 8
F32 = mybir.dt.float32
BF16 = mybir.dt.bfloat16
NT = S // 512     # 4 free-dim tiles of 512
TC = S // 128     # 16 t-chunks of 128

# CoreSim doesn't implement Erf; dev-only switch to validate logic in sim.
ERF_FUNC_NAME = "Tanh" if os.environ.get("KERNEL_SIM_TANH", "0") == "1" else "Erf"

LAST_EXEC_NS = None
LAST_RESULTS = None


def _split_multi_waits(nc: bass.Bass) -> None:
    """This walrus build rejects instructions carrying more than one sync
    wait (codegen 'Too many sync wait commands', hit by the Tile kernel-tail
    drain).  Hoist all but the last wait of any multi-wait instruction onto
    single-wait Drain instructions inserted just before it on the same
    engine — semantically identical, one wait per instruction."""
    for f in nc.m.functions:
        for b in f.blocks:
            new_insts = []
            changed = False
            for inst in b.instructions:
                si = inst.sync_info
                waits = list(si.on_wait) if si is not None and si.on_wait else []
                if len(waits) > 1:
                    changed = True
                    for n, w in enumerate(waits[:-1]):
                        d = mybir.InstDrain(
                            name=f"{inst.name}-wsplit{n}",
                            engine=inst.engine,
                            ins=[],
                            outs=[],
                            sync_info=mybir.SyncInfo(on_wait=[w], on_update=[]),
                        )
                        new_insts.append(d)
                    si.on_wait = [waits[-1]]
                new_insts.append(inst)
            if changed:
                b.instructions[:] = new_insts


def build_program() -> bass.Bass:
    nc = bass.Bass(target_bir_lowering=False, debug=False)

    x_t = nc.declare_dram_parameter("xt", [128, 6 * S], BF16, isOutput=False)
    wall = nc.declare_dram_parameter("wall", [128, 6 * 576], BF16, isOutput=False)
    wproj = nc.declare_dram_parameter("wproj", [HPC * HD, D], BF16, isOutput=False)
    csc = nc.declare_dram_parameter("csc", [128, S], BF16, isOutput=False)
    css = nc.declare_dram_parameter("css", [128, S], BF16, isOutput=False)
    swp = nc.declare_dram_parameter("swp", [128, 128], BF16, isOutput=False)
    out_d = nc.declare_dram_parameter("out", [S, D], F32, isOutput=True)

    erf_func = getattr(mybir.ActivationFunctionType, ERF_FUNC_NAME)
    # per-head row split of the AV output: YROW = y rows, DROW = denom rows.
    YROW = [slice(0, HD), slice(HD, 128), slice(0, HD)]
    DROW = [slice(HD, 128), slice(0, HD), slice(HD, 128)]
    OOFF = [HD, 0, HD]     # ones block offset within the head's 128 cols

    with TileContext(nc) as tc:
        with ExitStack() as ctx:
            const = ctx.enter_context(tc.tile_pool(name="const", bufs=1))
            tpool = ctx.enter_context(tc.tile_pool(name="tpool", bufs=3))
            wtp = ctx.enter_context(tc.tile_pool(name="wtp", bufs=5))
            nrm = ctx.enter_context(tc.tile_pool(name="nrm", bufs=2))
            ostp = ctx.enter_context(tc.tile_pool(name="ostp", bufs=3))

            # ---- input DMAs (host pre-packs xt/wall as [128, 6*...] so
            # one descriptor covers all six k-chunks) ----
            WA_all = const.tile([128, 6 * 576], BF16, tag="wa")
            nc.sync.dma_start(out=WA_all, in_=wall[:, :])
            WA = [WA_all[:, kc * 576:(kc + 1) * 576] for kc in range(6)]
            XT_all = const.tile([128, 6 * S], BF16, tag="xt")
            x3 = x_t.rearrange("p (k s) -> p k s", k=6)
            xt3 = XT_all.rearrange("p (k s) -> p k s", k=6)
            ns0 = slice(0, 512)
            nc.sync.dma_start(out=xt3[:, :, ns0], in_=x3[:, :, ns0])
            XT = [XT_all[:, kc * S:(kc + 1) * S] for kc in range(6)]
            SWP = const.tile([128, 128], BF16, tag="swp")
            nc.sync.dma_start(out=SWP, in_=swp[:, :])
            CSC = const.tile([128, S], BF16, tag="csc")
            nc.sync.dma_start(out=CSC, in_=csc[:, :])
            CSS = const.tile([128, S], BF16, tag="css")
            nc.sync.dma_start(out=CSS, in_=css[:, :])
            for ntb in range(1, NT):
                ns = slice(ntb * 512, (ntb + 1) * 512)
                nc.sync.dma_start(out=xt3[:, :, ns], in_=x3[:, :, ns])
            WPP = const.tile([128, D], BF16, tag="wpp")
            nc.sync.dma_start(out=WPP, in_=wproj[0:128, :])
            WP2 = const.tile([HD, D], BF16, tag="wp2")
            nc.sync.dma_start(out=WP2, in_=wproj[128:192, :])

            # VEXT: per chunk c, head h: 128 cols = [v|ones] (h0,h2), [ones|v] (h1)
            VEXT = const.tile([128, TC * 384], BF16, tag="vext")
            v4 = VEXT.rearrange("p (c h x) -> p c h x", c=TC, h=HPC, x=128)
            for h in range(HPC):
                nc.gpsimd.memset(v4[:, :, h, OOFF[h]:OOFF[h] + HD], 1.0)

            C = [const.tile([128, S], BF16, tag=f"c{m}", name=f"c{m}") for m in range(3)]
            R = [const.tile([128, S], BF16, tag=f"r{m}", name=f"r{m}") for m in range(3)]
            # k2 relocated to base partition 0 (matmul requires lhsT/rhs at
            # the same base partition; q2 is at rows 0:64 of R[2])
            K2 = const.tile([HD, S], BF16, tag="k2")
            # heads 0 (rows 0:64) and 1 (rows 64:128) share YT01 so the
            # projection can pair them into K=128 matmuls
            YT01 = const.tile([128, S], BF16, tag="yt01")
            YT2 = const.tile([HD, S], BF16, tag="yt2")
            YU = [const.tile([128, S], F32, tag=f"yu{h}", name=f"yu{h}") for h in range(HPC)]

            QS = [R[0][0:HD, :], R[0][HD:128, :], R[2][0:HD, :]]
            KS = [R[1][0:HD, :], R[1][HD:128, :], K2[:, :]]

            # ---- merged stream: per nt-block [QKV(nt), rope(nt), vT chunks,
            # attention(si=nt)]; AV matmuls lag LAG pairs behind scores ----
            LAG = 3
            with tc.tile_pool(name="mps", bufs=3, space="PSUM") as mps, \
                 tc.tile_pool(name="ypsp", bufs=2, space="PSUM") as ypsp:
                ypsmap = {}
                pend = []

                def emit_av(task, wt):
                    si, h, p, npair = task
                    key = (si, h)
                    if key not in ypsmap:
                        ypsmap[key] = ypsp.tile(
                            [128, 512], F32, tag="yps", name=f"yps{si}_{h}")
                    yps = ypsmap[key]
                    for half in range(2):
                        tcb = 2 * p + half
                        lo = max(0, tcb - 4 * si) * 128
                        nc.tensor.matmul(
                            yps[:, lo:512],
                            lhsT=VEXT[:, tcb * 384 + h * 128:tcb * 384 + (h + 1) * 128],
                            rhs=wt[:, half * 512 + lo:(half + 1) * 512],
                            start=(tcb == 0),
                            stop=(tcb == 2 * npair - 1),
                            skip_group_check=True,
                        )
                    if p == npair - 1:
                        ss = slice(si * 512, (si + 1) * 512)
                        nc.vector.tensor_copy(out=YU[h][:, ss], in_=yps)
                        del ypsmap[key]

                def qk_group(ntb, m):
                    ns = slice(ntb * 512, (ntb + 1) * 512)
                    ps = mps.tile([128, 1024], F32, tag="m", name=f"qk{m}_{ntb}")
                    for kc in range(6):
                        nc.tensor.matmul(
                            ps[:, 0:512],
                            lhsT=WA[kc][:, m * 128:(m + 1) * 128],
                            rhs=XT[kc][:, ns],
                            start=(kc == 0),
                            stop=(kc == 5),
                        )
                    nc.scalar.copy(out=C[m][:, ns], in_=ps[:, 0:512])

                def sw_group(ntb, m):
                    ns = slice(ntb * 512, (ntb + 1) * 512)
                    sw = mps.tile([128, 1024], F32, tag="m", name=f"sw{m}_{ntb}")
                    nc.tensor.matmul(
                        sw[:, 0:512], lhsT=SWP, rhs=C[m][:, ns],
                        start=True, stop=True,
                    )
                    t1 = tpool.tile([128, 512], BF16, tag="t1")
                    t2 = tpool.tile([128, 512], BF16, tag="t2")
                    nc.vector.tensor_mul(t1, C[m][:, ns], CSC[:, ns])
                    nc.vector.tensor_mul(t2, sw[:, 0:512], CSS[:, ns])
                    nc.vector.tensor_add(R[m][:, ns], t1, t2)
                    if m == 2:
                        nc.sync.dma_start(out=K2[:, ns], in_=R[2][HD:128, ns])

                def vt_group(tcb):
                    ts = slice(tcb * 128, (tcb + 1) * 128)
                    ps = mps.tile([128, 1024], F32, tag="m", name=f"vt{tcb}")
                    for kc in range(6):
                        nc.tensor.matmul(
                            ps[:, 0:HPC * HD],
                            lhsT=XT[kc][:, ts],
                            rhs=WA[kc][:, 384:576],
                            start=(kc == 0),
                            stop=(kc == 5),
                        )
                    base = tcb * 384
                    # v0 -> cols [0:64); v1,v2 -> contiguous cols [192:320)
                    nc.scalar.copy(out=VEXT[:, base:base + HD], in_=ps[:, 0:HD])
                    nc.scalar.copy(
                        out=VEXT[:, base + 192:base + 320], in_=ps[:, HD:3 * HD])

                def block_groups(ntb):
                    gs = []
                    for m in range(3):
                        gs.append(lambda m=m: qk_group(ntb, m))
                    for m in range(3):
                        gs.append(lambda m=m: sw_group(ntb, m))
                    for tcb in range(4 * ntb, 4 * ntb + 4):
                        gs.append(lambda tcb=tcb: vt_group(tcb))
                    return gs

                def emit_pair(si, h, p, npair):
                    sc = mps.tile([128, 1024], F32, tag="m", name=f"sc{si}_{h}_{p}")
                    for half in range(2):
                        tcb = 2 * p + half
                        lo = max(0, tcb - 4 * si) * 128
                        nc.tensor.matmul(
                            sc[:, half * 512 + lo:(half + 1) * 512],
                            lhsT=KS[h][:, tcb * 128:(tcb + 1) * 128],
                            rhs=QS[h][:, si * 512 + lo:(si + 1) * 512],
                            start=True,
                            stop=True,
                        )
                    wt = wtp.tile([128, 1024], BF16, tag="wt")
                    if p >= 2 * si:
                        # band pair: erf only the valid column ranges
                        for half in range(2):
                            tcb = 2 * p + half
                            lo = max(0, tcb - 4 * si) * 128
                            nc.scalar.activation(
                                out=wt[:, half * 512 + lo:(half + 1) * 512],
                                in_=sc[:, half * 512 + lo:(half + 1) * 512],
                                func=erf_func, scale=0.125,
                            )
                    else:
                        nc.scalar.activation(
                            out=wt, in_=sc, func=erf_func, scale=0.125)
                    nc.vector.tensor_scalar_add(wt, wt, 1.0)
                    if p >= 2 * si:  # diagonal band: causal mask
                        for half in range(2):
                            tcb = 2 * p + half
                            lo = max(0, tcb - 4 * si) * 128
                            nc.gpsimd.affine_select(
                                out=wt[:, half * 512 + lo:(half + 1) * 512],
                                in_=wt[:, half * 512 + lo:(half + 1) * 512],
                                compare_op=mybir.AluOpType.is_ge,
                                fill=0.0,
                                base=0,
                                channel_multiplier=-1,
                                pattern=[[1, 512 - lo]],
                            )
                    pend.append(((si, h, p, npair), wt))
                    if len(pend) > LAG:
                        emit_av(*pend.pop(0))

                # block 0 up front; block si+1 rationed between attention
                # pairs of si as dependency-free PE filler (keeps the PE
                # queue stocked so the DVFS p-state stays at 2.4 GHz)
                for g in block_groups(0):
                    g()
                for si in range(NT):
                    npair = 2 * (si + 1)
                    fillers = block_groups(si + 1) if si + 1 < NT else []
                    pairs = [(si, h, p, npair) for h in range(HPC) for p in range(npair)]
                    fi = 0
                    for i, (si_, h, p, npair_) in enumerate(pairs):
                        emit_pair(si_, h, p, npair_)
                        want = (i + 1) * len(fillers) // len(pairs)
                        while fi < want:
                            fillers[fi]()
                            fi += 1
                while pend:
                    emit_av(*pend.pop(0))

            # ---- tail: normalize (2 table loads total) + projection ----
            with tc.tile_pool(name="pop", bufs=6, space="PSUM") as pop:
                LG = [nrm.tile([128, S], F32, tag="lg", name=f"lg{h}", bufs=3)
                      for h in range(HPC)]
                for h in range(HPC):
                    nc.scalar.activation(
                        out=LG[h][DROW[h], :], in_=YU[h][DROW[h], :],
                        func=mybir.ActivationFunctionType.Ln,
                    )
                for h in range(HPC):
                    rs = nrm.tile([128, S], F32, tag="rs", name=f"rs{h}")
                    nc.scalar.activation(
                        out=rs[DROW[h], :], in_=LG[h][DROW[h], :],
                        func=mybir.ActivationFunctionType.Exp, scale=-1.0,
                    )
                    rl = nrm.tile([128, S], F32, tag="rl", name=f"rl{h}")
                    nc.sync.dma_start(out=rl[YROW[h], :], in_=rs[DROW[h], :])
                    ydst = YT01[YROW[h], :] if h < 2 else YT2[:, :]
                    nc.vector.tensor_mul(ydst, YU[h][YROW[h], :], rl[YROW[h], :])

                for sci in range(TC):
                    scs = slice(sci * 128, (sci + 1) * 128)
                    po1 = pop.tile([128, 512], F32, tag="po")
                    po2 = pop.tile([128, 512], F32, tag="po")
                    nc.tensor.matmul(
                        po1, lhsT=YT01[:, scs], rhs=WPP[:, 0:512],
                        start=True, stop=False,
                    )
                    nc.tensor.matmul(
                        po1, lhsT=YT2[:, scs], rhs=WP2[:, 0:512],
                        start=False, stop=True,
                    )
                    nc.tensor.matmul(
                        po2[:, 0:256], lhsT=YT01[:, scs], rhs=WPP[:, 512:768],
                        start=True, stop=False,
                    )
                    nc.tensor.matmul(
                        po2[:, 0:256], lhsT=YT2[:, scs], rhs=WP2[:, 512:768],
                        start=False, stop=True,
                    )
                    ost = ostp.tile([128, D], F32, tag="ost")
                    if sci % 2 == 0:
                        nc.scalar.copy(out=ost[:, 0:512], in_=po1)
                        nc.vector.tensor_copy(out=ost[:, 512:768], in_=po2[:, 0:256])
                    else:
                        nc.vector.tensor_copy(out=ost[:, 0:512], in_=po1)
                        nc.scalar.copy(out=ost[:, 512:768], in_=po2[:, 0:256])
                    nc.sync.dma_start(out=out_d[scs, :], in_=ost)

    return nc


_PROGRAM = None


def _get_program() -> bass.Bass:
    global _PROGRAM
    if _PROGRAM is None:
        _PROGRAM = build_program()
        _split_multi_waits(_PROGRAM)
    return _PROGRAM


def _bf16(arr):
    return np.ascontiguousarray(arr).astype(mybir.dt.np(BF16))


def make_in_maps(x, Wq, Wk, Wv, Wproj):
    x = np.asarray(x, dtype=np.float32)
    Wq = np.asarray(Wq, dtype=np.float32)
    Wk = np.asarray(Wk, dtype=np.float32)
    Wv = np.asarray(Wv, dtype=np.float32)
    Wproj = np.asarray(Wproj, dtype=np.float32)

    half = HD // 2
    j = np.arange(half, dtype=np.float64)
    freq = 1.0 / (10000.0 ** (j / half))
    ang = np.arange(S, dtype=np.float64)[None, :] * freq[:, None]   # [32, S]
    cosT = np.cos(ang).astype(np.float32)
    sinT = np.sin(ang).astype(np.float32)
    csc = np.tile(np.vstack([cosT, cosT]), (2, 1))                  # [128, S]
    css = np.tile(np.vstack([-sinT, sinT]), (2, 1))

    swp = np.zeros((128, 128), dtype=np.float32)
    for blk in range(2):
        for jj in range(half):
            swp[blk * 64 + jj, blk * 64 + half + jj] = 1.0
            swp[blk * 64 + half + jj, blk * 64 + jj] = 1.0

    perm = np.concatenate([np.arange(0, HD, 2), np.arange(1, HD, 2)])

    in_maps = []
    for c in range(NCORES):
        b = c // 4
        hs = [(c % 4) * HPC + i for i in range(HPC)]
        rq = [Wq[h * HD:(h + 1) * HD][perm, :] for h in hs]
        rk = [Wk[h * HD:(h + 1) * HD][perm, :] for h in hs]
        rv = [Wv[h * HD:(h + 1) * HD, :] for h in hs]
        cols = np.concatenate(
            [rq[0], rq[1], rk[0], rk[1], rq[2], rk[2], rv[0], rv[1], rv[2]],
            axis=0,
        )                                                           # [576, D]
        wallm = np.ascontiguousarray(cols.T)                        # [D, 576]
        dims = np.concatenate([np.arange(h * HD, (h + 1) * HD) for h in hs])
        wproj_t = np.ascontiguousarray(Wproj[:, dims].T)            # [192, D]
        xt = x[b].T                                                 # [D, S]
        xt2 = np.ascontiguousarray(
            xt.reshape(6, 128, S).transpose(1, 0, 2).reshape(128, 6 * S))
        wall2 = np.ascontiguousarray(
            wallm.reshape(6, 128, 576).transpose(1, 0, 2).reshape(128, 6 * 576))
        in_maps.append({
            "xt": _bf16(xt2),
            "wall": _bf16(wall2),
            "wproj": _bf16(wproj_t),
            "csc": _bf16(csc),
            "css": _bf16(css),
            "swp": _bf16(swp),
        })
    return in_maps


def kernel(x, Wq, Wk, Wv, Wproj):
    global LAST_EXEC_NS, LAST_RESULTS
    nc = _get_program()
    in_maps = make_in_maps(x, Wq, Wk, Wv, Wproj)
    trace = os.environ.get("KERNEL_TRACE", "0") == "1"
    res = run_bass_kernel_spmd(nc, in_maps, list(range(NCORES)), trace=trace)
    LAST_EXEC_NS = res.exec_time_ns
    LAST_RESULTS = res
    outs = [np.asarray(r["out"], dtype=np.float32) for r in res.results]
    out = np.empty((2, S, D), dtype=np.float32)
    out[0] = outs[0] + outs[1] + outs[2] + outs[3]
    out[1] = outs[4] + outs[5] + outs[6] + outs[7]
    return out
